# revision 2
# baseline (speedup 1.0000x reference)
"""Bottleneck residual block (1x1 -> 3x3 -> 1x1 conv + BN + residual) on 8 NeuronCores.

Strategy: pure data-parallel over the batch dim (16 images -> 2 per core).
All convs are exact-integer arithmetic in disguise (int8 activations x
small power-of-two int weights, values << 2^24), so every matmul runs in
bf16 with fp32 PSUM accumulation and stays bit-exact.  The BN + round +
clip + relu chain is reproduced bit-exactly with per-partition
scale/bias ops and the +/-1.5*2^23 magic-number trick for
round-half-to-even.

Shapes are hardcoded for N=16, Cin=Cout=1024, width=256, H=W=28.
"""

import numpy as np
import ml_dtypes

BF16 = ml_dtypes.bfloat16
M15 = np.float32(12582912.0)  # 1.5 * 2^23: RNE magic constant for |t| < 2^22

N_CORES = 8
N_PER_CORE = 2          # images per core
HW = 28 * 28            # 784 spatial positions per image
F = N_PER_CORE * HW     # 1568 free-dim elements per core
FB = 392                # matmul free-dim block (14 rows of 28)

_CACHE = {}


def _build():
    """Build + compile the per-core Bass kernel once per process."""
    import concourse.bacc as bacc
    import concourse.mybir as mybir
    import concourse.tile as tile

    dt = mybir.dt
    f32, bf16, i8 = dt.float32, dt.bfloat16, dt.int8
    Alu = mybir.AluOpType
    Act = mybir.ActivationFunctionType

    nc = bacc.Bacc("TRN2", target_bir_lowering=False, debug=False, num_devices=N_CORES)

    x_d = nc.dram_tensor("x", [8, 128, F], bf16, kind="ExternalInput")
    w1_d = nc.dram_tensor("w1", [8, 2, 128, 128], bf16, kind="ExternalInput")
    w2_d = nc.dram_tensor("w2", [9, 2, 2, 128, 128], bf16, kind="ExternalInput")
    w3_d = nc.dram_tensor("w3", [2, 8, 128, 128], bf16, kind="ExternalInput")
    vec_d = nc.dram_tensor("vec", [128, 24], f32, kind="ExternalInput")
    out_d = nc.dram_tensor("out", [8, 128, F], i8, kind="ExternalOutput")

    with tile.TileContext(nc) as tc:
        with (
            tc.tile_pool(name="persist", bufs=1) as pp,
            tc.tile_pool(name="stage", bufs=3) as sp,
            tc.tile_pool(name="psum", bufs=8, space="PSUM") as psp,
        ):
            # ---- persistent SBUF tensors + input DMA ----
            vec_sb = pp.tile([128, 24], f32, tag="vec", name="vec")
            nc.sync.dma_start(vec_sb[:], vec_d[:])

            x_sb = []
            for k in range(8):
                t = pp.tile([128, F], bf16, tag=f"x{k}", name=f"x{k}")
                nc.sync.dma_start(t[:], x_d[k])
                x_sb.append(t)

            w1_sb = [[pp.tile([128, 128], bf16, tag=f"w1_{k}_{m}", name=f"w1_{k}_{m}") for m in range(2)]
                     for k in range(8)]
            for k in range(8):
                for m in range(2):
                    nc.sync.dma_start(w1_sb[k][m][:], w1_d[k, m])
            w2_sb = [[[pp.tile([128, 128], bf16, tag=f"w2_{t9}_{k}_{m}", name=f"w2_{t9}_{k}_{m}") for m in range(2)]
                      for k in range(2)] for t9 in range(9)]
            for t9 in range(9):
                for k in range(2):
                    for m in range(2):
                        nc.sync.dma_start(w2_sb[t9][k][m][:], w2_d[t9, k, m])
            w3_sb = [[pp.tile([128, 128], bf16, tag=f"w3_{k}_{m}", name=f"w3_{k}_{m}") for m in range(8)]
                     for k in range(2)]
            for k in range(2):
                for m in range(8):
                    nc.sync.dma_start(w3_sb[k][m][:], w3_d[k, m])

            # stage-1 output, zero-padded to 30x30 per image for the 3x3 conv
            s1p = [pp.tile([128, 2, 30, 30], bf16, tag=f"s1p{m}", name=f"s1p{m}") for m in range(2)]
            for m in range(2):
                nc.gpsimd.memset(s1p[m][:], 0.0)
            s2 = [pp.tile([128, F], bf16, tag=f"s2_{m}", name=f"s2_{m}") for m in range(2)]
            out_sb = [pp.tile([128, F], i8, tag=f"o{m}", name=f"o{m}") for m in range(8)]

            # per-channel scale/bias column views  (a' = alpha*2^-12, b' = beta*2^q)
            a1 = [vec_sb[:, m:m + 1] for m in range(2)]
            b1 = [vec_sb[:, 2 + m:3 + m] for m in range(2)]
            a2 = [vec_sb[:, 4 + m:5 + m] for m in range(2)]
            b2 = [vec_sb[:, 6 + m:7 + m] for m in range(2)]
            a3 = [vec_sb[:, 8 + m:9 + m] for m in range(8)]
            b3 = [vec_sb[:, 16 + m:17 + m] for m in range(8)]

            # ---- stage 1: 1x1 conv (K=1024 -> M=256), relu(round(a*c+b)) ----
            for m in range(2):
                t = sp.tile([128, F], f32, tag="t", name="t")
                for fb in range(4):
                    ps = psp.tile([128, FB], f32, tag="ps", name="ps")
                    for kt in range(8):
                        nc.tensor.matmul(
                            ps[:], w1_sb[kt][m][:], x_sb[kt][:, fb * FB:(fb + 1) * FB],
                            start=(kt == 0), stop=(kt == 7))
                    # t = fl(a' * c)   (exact single-rounding product)
                    nc.scalar.activation(t[:, fb * FB:(fb + 1) * FB], ps[:],
                                         Act.Copy, bias=0.0, scale=a1[m])
                # t = fl(fl(t + b') + M15)  -> RNE(t) + M15
                nc.vector.tensor_scalar(t[:], t[:], b1[m], float(M15), Alu.add, Alu.add)
                # s1 = max(t - M15, 0), scattered into the padded interior
                nc.vector.tensor_scalar(s1p[m][:, :, 1:29, 1:29], t[:],
                                        float(M15), 0.0, Alu.subtract, Alu.max)

            # ---- stage 2: 3x3 conv (K=256 -> M=256), same epilogue ----
            for m in range(2):
                t = sp.tile([128, F], f32, tag="t", name="t")
                for n in range(2):
                    for hb in range(2):
                        fb = n * 2 + hb
                        h0 = hb * 14
                        ps = psp.tile([128, FB], f32, tag="ps", name="ps")
                        first = True
                        for kt in range(2):
                            for dy in range(3):
                                for dx in range(3):
                                    rhs = s1p[kt][:, n, h0 + dy:h0 + dy + 14, dx:dx + 28]
                                    nc.tensor.matmul(
                                        ps[:], w2_sb[dy * 3 + dx][kt][m][:], rhs,
                                        start=first,
                                        stop=(kt == 1 and dy == 2 and dx == 2))
                                    first = False
                        nc.scalar.activation(t[:, fb * FB:(fb + 1) * FB], ps[:],
                                             Act.Copy, bias=0.0, scale=a2[m])
                nc.vector.tensor_scalar(t[:], t[:], b2[m], float(M15), Alu.add, Alu.add)
                nc.vector.tensor_scalar(s2[m][:], t[:], float(M15), 0.0,
                                        Alu.subtract, Alu.max)

            # ---- stage 3: 1x1 conv (K=256 -> M=1024) + residual + clamp ----
            for m in range(8):
                t = sp.tile([128, F], f32, tag="t", name="t")
                for fb in range(4):
                    ps = psp.tile([128, FB], f32, tag="ps", name="ps")
                    for kt in range(2):
                        nc.tensor.matmul(
                            ps[:], w3_sb[kt][m][:], s2[kt][:, fb * FB:(fb + 1) * FB],
                            start=(kt == 0), stop=(kt == 1))
                    nc.scalar.activation(t[:, fb * FB:(fb + 1) * FB], ps[:],
                                         Act.Copy, bias=0.0, scale=a3[m])
                nc.vector.tensor_scalar(t[:], t[:], b3[m], float(M15), Alu.add, Alu.add)
                r = sp.tile([128, F], bf16, tag="r", name="r")
                nc.vector.tensor_scalar(r[:], t[:], float(M15), None, Alu.subtract)
                nc.vector.tensor_tensor(r[:], r[:], x_sb[m][:], Alu.add)
                nc.vector.tensor_scalar(out_sb[m][:], r[:], 0.0, 127.0,
                                        Alu.max, Alu.min)
                nc.sync.dma_start(out_d[m], out_sb[m][:])

    nc.compile()
    return nc


def _get_nc():
    if "nc" not in _CACHE:
        _CACHE["nc"] = _build()
    return _CACHE["nc"]


def _pack_inputs(inputs):
    """Host-side: effective weights, per-core shards, bf16 casts."""
    f32 = np.float32
    x = np.asarray(inputs["x"])

    def eff(w2, s):
        return (np.asarray(w2, dtype=f32) *
                np.exp2(np.asarray(s).astype(f32))).astype(f32)

    # lhsT tiles: [K, M] per (ktile, mtile)
    w1 = eff(inputs["w2_1"], inputs["s1"])[:, :, 0, 0]          # [256, 1024]
    w1 = np.ascontiguousarray(
        w1.T.reshape(8, 128, 2, 128).transpose(0, 2, 1, 3)).astype(BF16)
    w2e = eff(inputs["w2_2"], inputs["s2"])                      # [256, 256, 3, 3]
    w2 = np.stack([
        np.ascontiguousarray(
            w2e[:, :, dy, dx].T.reshape(2, 128, 2, 128).transpose(0, 2, 1, 3))
        for dy in range(3) for dx in range(3)
    ]).astype(BF16)                                              # [9, 2, 2, 128, 128]
    w3 = eff(inputs["w2_3"], inputs["s3"])[:, :, 0, 0]           # [1024, 256]
    w3 = np.ascontiguousarray(
        w3.T.reshape(2, 128, 8, 128).transpose(0, 2, 1, 3)).astype(BF16)

    vec = np.zeros((128, 24), dtype=f32)
    scl = np.exp2(f32(-12.0))
    for m in range(2):
        sl = slice(m * 128, (m + 1) * 128)
        vec[:, m] = np.asarray(inputs["alpha1"], dtype=f32)[sl] * scl
        vec[:, 2 + m] = (np.asarray(inputs["beta1"], dtype=f32)[sl] *
                         np.exp2(np.asarray(inputs["q1"]).astype(f32)[sl]))
        vec[:, 4 + m] = np.asarray(inputs["alpha2"], dtype=f32)[sl] * scl
        vec[:, 6 + m] = (np.asarray(inputs["beta2"], dtype=f32)[sl] *
                         np.exp2(np.asarray(inputs["q2"]).astype(f32)[sl]))
    for m in range(8):
        sl = slice(m * 128, (m + 1) * 128)
        vec[:, 8 + m] = np.asarray(inputs["alpha3"], dtype=f32)[sl] * scl
        vec[:, 16 + m] = (np.asarray(inputs["beta3"], dtype=f32)[sl] *
                          np.exp2(np.asarray(inputs["q3"]).astype(f32)[sl]))

    in_maps = []
    for c in range(N_CORES):
        xc = x[c * N_PER_CORE:(c + 1) * N_PER_CORE]              # [2, 1024, 28, 28]
        xc = np.ascontiguousarray(
            xc.transpose(1, 0, 2, 3).reshape(8, 128, F)).astype(BF16)
        in_maps.append({"x": xc, "w1": w1, "w2": w2, "w3": w3, "vec": vec})
    return in_maps


def _assemble(results):
    outs = []
    for c in range(N_CORES):
        o = results[c]["out"]                                    # [8, 128, 1568] int8
        o = o.reshape(1024, N_PER_CORE, 28, 28).transpose(1, 0, 2, 3)
        outs.append(o)
    return np.concatenate(outs, axis=0).astype(np.float32)


def _run(inputs, trace=False, **kwargs):
    from concourse.bass_utils import run_bass_kernel_spmd
    nc = _get_nc()
    in_maps = _pack_inputs(inputs)
    res = run_bass_kernel_spmd(nc, in_maps, list(range(N_CORES)),
                               trace=trace, **kwargs)
    return _assemble(res.results), res


def kernel(**inputs):
    out, _ = _run(inputs)
    return out
